# revision 1
# baseline (speedup 1.0000x reference)
"""Multi-head attention (LoRA QKV + ALiBi + causal softmax + output proj) on 8 TRN2 cores.

Sharding: core = (batch b in 0..3, head-group hg in 0..1); each core handles one batch
element and 8 of the 16 heads.  LoRA is folded into effective weights on the host
(W_eff = W + 2*A@B, exact algebra).  Each core computes a partial projection output
(its 512 attention dims x full Wp rows); the host sums the two partials per batch.

On-core math (all matmuls in float32r = full PE speed for free-dim >= 256):
  qT[d,t] = sum_e wqT[e,d] * xT[e,t]          (wqT pre-scaled by 1/sqrt(dh) on host)
  kT[d,t], v[t,d] similar
  sT[j,i] = sum_d kT[d,j] qT[d,i]             (two heads packed per 64-row PE strip)
  p[j,i]  = exp(sT[j,i] - slope*j - C)        (analytic softmax max M_i = slope*i + C
                                               cancels the +slope*i ALiBi term; exact
                                               after normalization)
  causal: p[j,i] = 0 where j > i              (gpsimd affine_select on diagonal tiles)
  pv[d,i] = sum_j v'[j,d] p[j,i]              (v' has a ones column -> row d=64 is the
                                               softmax denominator)
  outT[d,i] = pv[d,i] / pv[64,i]              (reciprocal + ones-matmul broadcast)
  out[t,e] = sum_d outT[d,t] * wpT[d,e]       (partial; host adds the other head-group)

Schedule: V for all heads first; then per head-pair hp the attention c-chunks are
explicitly interleaved with the NEXT pair's qT/kT projection groups, because the PE
executes its stream in order and ACT (exp) is the attention-phase bottleneck: the
projection matmuls fill PE while ACT drains.
"""

import math
from contextlib import ExitStack

import numpy as np

import concourse.bacc as bacc
import concourse.mybir as mybir
import concourse.tile as tile
from concourse.bass_utils import run_bass_kernel_spmd

T, E, DH, H = 2048, 1024, 64, 16
HL = 8              # heads per core
NKT = 8             # contraction tiles of 128 over E
NTT = 16            # token tiles of 128 over T
CB = 12.0           # safety constant in the analytic softmax max
NEG = -1.0e30

_NC_CACHE = None


def _build_nc():
    f32 = mybir.dt.float32
    f32r = mybir.dt.float32r
    Exp = mybir.ActivationFunctionType.Exp

    nc = bacc.Bacc(trn_type="TRN2", target_bir_lowering=False, debug=False)
    xT_d = nc.declare_dram_parameter("xT", [E, T], f32, isOutput=False)
    wqT_d = nc.declare_dram_parameter("wqT", [E, 512], f32, isOutput=False)
    wkT_d = nc.declare_dram_parameter("wkT", [E, 512], f32, isOutput=False)
    wvT_d = nc.declare_dram_parameter("wvT", [E, 512], f32, isOutput=False)
    wpT_d = nc.declare_dram_parameter("wpT", [512, E], f32, isOutput=False)
    eb_d = nc.declare_dram_parameter("ebias", [128, 128], f32, isOutput=False)
    tri_d = nc.declare_dram_parameter("trineg", [128, 128], f32, isOutput=False)
    ones_d = nc.declare_dram_parameter("onesd", [128, 128], f32, isOutput=False)
    out_d = nc.declare_dram_parameter("out", [T, E], f32, isOutput=True)

    with ExitStack() as st:
        tc = st.enter_context(tile.TileContext(nc))
        ps = st.enter_context(tc.tile_pool(name="ps", bufs=1, space="PSUM"))
        # psum tags: acc(2) + s(4) + pv(2) = 8 banks exactly
        sb_r = st.enter_context(tc.tile_pool(name="sbr", bufs=1, side="right"))
        sb_x = st.enter_context(tc.tile_pool(name="sbx", bufs=1, side="left"))
        sb_l = st.enter_context(tc.tile_pool(name="sbl", bufs=1, side="left"))

        # ---------- DMA plumbing ----------
        # sync queue: xT column-chunks paced against the V stage; gpsimd queue:
        # weights + small constants, so they don't delay the xT stream.
        xts = []
        for k in range(NKT):
            xts.append(sb_x.tile([128, T], f32r, tag=f"xt{k}", bufs=1, name=f"xt{k}"))

        def dma_xt_chunk(ck):
            for k in range(NKT):
                nc.sync.dma_start(
                    out=xts[k][:, ck * 512:(ck + 1) * 512],
                    in_=xT_d[k * 128:(k + 1) * 128, ck * 512:(ck + 1) * 512].bitcast(f32r))

        dma_xt_chunk(0)
        dma_xt_chunk(1)
        wvs = []
        for k in range(NKT):
            t = sb_l.tile([128, 512], f32r, tag="wst", bufs=8, name=f"wv{k}")
            nc.gpsimd.dma_start(out=t[:], in_=wvT_d[k * 128:(k + 1) * 128, :].bitcast(f32r))
            wvs.append(t)
        gv_sb = sb_r.tile([128, 128], f32, tag="gv", bufs=1)
        ones_t = sb_r.tile([128, 64], f32r, tag="ones", bufs=1)

        qts = [None] * 4
        kts = [None] * 4
        wqk = [None] * 4
        outTs = [None] * 4

        def emit_wqk_dma(hp):
            tiles = {}
            for which, wd in (("q", wqT_d), ("k", wkT_d)):
                wt = sb_l.tile([128, 1024], f32r, tag="wqk", bufs=2,
                               name=f"w{which}{hp}")
                src = wd[:, hp * 128:(hp + 1) * 128]
                src = src.rearrange("(k p) m -> p k m", p=128).bitcast(f32r)
                nc.gpsimd.dma_start(out=wt.rearrange("p (k m) -> p k m", k=NKT), in_=src)
                tiles[which] = wt
            wqk[hp] = tiles
            qts[hp] = sb_l.tile([128, T], f32r, tag="qt", bufs=2, name=f"qt{hp}")
            kts[hp] = sb_l.tile([128, T], f32r, tag="kt", bufs=2, name=f"kt{hp}")

        def emit_qk_group(hp, which, tck):
            wt = wqk[hp][which]
            ot = qts[hp] if which == "q" else kts[hp]
            pq = ps.tile([128, 512], f32, tag="acc", bufs=2)
            for k in range(NKT):
                nc.tensor.matmul(pq[:], wt[:, k * 128:(k + 1) * 128],
                                 xts[k][:, tck * 512:(tck + 1) * 512],
                                 start=(k == 0), stop=(k == NKT - 1))
            nc.vector.tensor_copy(ot[:, tck * 512:(tck + 1) * 512], pq[:])

        vts = [None] * NTT

        def emit_v_group(tt):
            pvm = ps.tile([128, 512], f32, tag="acc", bufs=2)
            for k in range(NKT):
                nc.tensor.matmul(pvm[:], xts[k][:, tt * 128:(tt + 1) * 128], wvs[k][:],
                                 start=(k == 0), stop=(k == NKT - 1))
            vt = sb_r.tile([128, HL * 65], f32r, tag=f"v{tt}", bufs=1, name=f"v{tt}")
            v3 = vt.rearrange("p (h c) -> p h c", h=HL)
            for h in range(HL):
                nc.vector.tensor_scalar_mul(
                    v3[:, h, 0:64], pvm[:, h * 64:(h + 1) * 64],
                    gv_sb[:, tt * HL + h:tt * HL + h + 1])
            nc.vector.tensor_copy(
                v3[:, :, 64:65],
                gv_sb[:, tt * HL:(tt + 1) * HL].rearrange("p (h c) -> p h c", c=1))
            vts[tt] = vt

        wps = [None] * 8

        def emit_wp_dma():
            for i in range(8):  # i = hp*2 + ec
                hp, ec = i // 2, i % 2
                t = sb_l.tile([128, 512], f32r, tag="wst", bufs=8, name=f"wp{i}")
                nc.gpsimd.dma_start(
                    out=t[:],
                    in_=wpT_d[hp * 128:(hp + 1) * 128,
                              ec * 512:(ec + 1) * 512].bitcast(f32r))
                wps[i] = t

        def emit_proj_group(tt, ec):
            po = ps.tile([128, 512], f32, tag="acc", bufs=2)
            for hp in range(4):
                nc.tensor.matmul(po[:], outTs[hp][:, tt * 128:(tt + 1) * 128],
                                 wps[hp * 2 + ec][:], start=(hp == 0), stop=(hp == 3))
            ob = sb_l.tile([128, 512], f32, tag="ob", bufs=2)
            nc.vector.tensor_copy(ob[:], po[:])
            nc.sync.dma_start(out=out_d[tt * 128:(tt + 1) * 128,
                                        ec * 512:(ec + 1) * 512],
                              in_=ob[:])

        # ---------- filler schedule: PE work emitted between attention chunks ----
        # deadline rule: attn(hp, c) needs q_hp[c], k_hp[0..c], v[0..4c+4)
        def fill_v(tts):
            return [lambda tt=tt: emit_v_group(tt) for tt in tts]

        def fill_qk(hp, tck):
            return [lambda: emit_qk_group(hp, "q", tck),
                    lambda: emit_qk_group(hp, "k", tck)]

        fills = {}
        fills[(0, 0)] = ([lambda: dma_xt_chunk(2)] + fill_v(range(4, 8))
                         + fill_qk(0, 2))
        fills[(0, 1)] = ([lambda: dma_xt_chunk(3)] + fill_v(range(8, 12))
                         + fill_qk(0, 3))
        fills[(0, 2)] = (fill_v(range(12, 16)) + [lambda: emit_wqk_dma(1)]
                         + fill_qk(1, 0) + [emit_wp_dma])
        fills[(0, 3)] = fill_qk(1, 1)
        fills[(1, 0)] = fill_qk(1, 2)
        fills[(1, 1)] = fill_qk(1, 3)
        fills[(1, 2)] = [lambda: emit_wqk_dma(2)] + fill_qk(2, 0)
        fills[(1, 3)] = fill_qk(2, 1)
        fills[(2, 0)] = fill_qk(2, 2)
        fills[(2, 1)] = fill_qk(2, 3)
        fills[(2, 2)] = [lambda: emit_wqk_dma(3)] + fill_qk(3, 0)
        fills[(2, 3)] = fill_qk(3, 1)

        def proj_fills(c):
            return [(lambda tt=tt, ec=ec: emit_proj_group(tt, ec))
                    for tt in range(4 * c, 4 * c + 4) for ec in range(2)]

        fills[(3, 0)] = fill_qk(3, 2) + proj_fills(0)
        fills[(3, 1)] = fill_qk(3, 3) + proj_fills(1)
        fills[(3, 2)] = proj_fills(2)
        fills[(3, 3)] = proj_fills(3)

        # ---------- preloop ----------
        nc.gpsimd.dma_start(out=gv_sb[:], in_=eb_d[:])
        nc.gpsimd.dma_start(out=ones_t[:], in_=ones_d[:, 0:64].bitcast(f32r))
        for tt in range(4):
            emit_v_group(tt)
        emit_wqk_dma(0)
        for fn in fill_qk(0, 0) + fill_qk(0, 1):
            fn()

        # ---------- attention ----------
        for hp in range(4):
            qt, kt = qts[hp], kts[hp]
            h0, h1 = 2 * hp, 2 * hp + 1
            oT = sb_r.tile([128, T], f32r, tag=f"ot{hp}", bufs=1, name=f"ot{hp}")
            outTs[hp] = oT
            for slot, c in enumerate(range(4)):
                pv0 = ps.tile([128, 512], f32, tag="pv", bufs=2)
                pv1 = ps.tile([128, 512], f32, tag="pv", bufs=2)
                njt = 4 * c + 4
                for jt in range(njt):
                    r = jt - 4 * c
                    # keep the moving free-dim >= 256 (fp32r runs 4x slower
                    # below 256): widen the r=3 chunk; extra columns are
                    # fully masked by a wider affine_select window
                    cw = max(512 - 128 * r, 256) if r > 0 else 512
                    mw = 128 * r - (512 - cw) + 128 if r > 0 else 128
                    ioff = c * 512 + (512 - cw)
                    s01 = ps.tile([128, 1024], f32, tag="s", bufs=2)
                    nc.tensor.matmul(s01[:, 0:cw], kt[0:64, jt * 128:(jt + 1) * 128],
                                     qt[0:64, ioff:ioff + cw], start=True, stop=True)
                    nc.tensor.matmul(s01[:, 512:512 + cw],
                                     kt[64:128, jt * 128:(jt + 1) * 128],
                                     qt[64:128, ioff:ioff + cw], start=True, stop=True)
                    p01 = sb_l.tile([128, 1024], f32r, tag="pt", bufs=2)
                    s3 = s01.rearrange("p (h m) -> p h m", h=2)
                    p3 = p01.rearrange("p (h m) -> p h m", h=2)
                    nc.scalar.activation(p3[:, :, 0:cw], s3[:, :, 0:cw], Exp)
                    if r >= 0:
                        # zero the j > i region at the head of the window:
                        # keep where (i - j) = (m - (mw - 128)) - pj >= 0
                        for off in (0, 512):
                            nc.gpsimd.affine_select(
                                out=p01[:, off:off + mw], in_=p01[:, off:off + mw],
                                compare_op=mybir.AluOpType.is_ge, fill=0.0,
                                base=-(mw - 128), pattern=[[1, mw]],
                                channel_multiplier=-1)
                    nc.tensor.matmul(pv0[0:65, 512 - cw:512],
                                     vts[jt][:, h0 * 65:h0 * 65 + 65], p01[:, 0:cw],
                                     start=(jt == 0), stop=(jt == njt - 1))
                    nc.tensor.matmul(pv1[0:65, 512 - cw:512],
                                     vts[jt][:, h1 * 65:h1 * 65 + 65],
                                     p01[:, 512:512 + cw],
                                     start=(jt == 0), stop=(jt == njt - 1))
                # normalize: outT[d, i] = pv[d, i] * (1 / pv[64, i])
                for par, pvx in ((0, pv0), (1, pv1)):
                    rr = sb_l.tile([65, 512], f32r, tag="rr", bufs=2)
                    with nc.allow_low_precision("f32r reciprocal of softmax denom"):
                        nc.vector.reciprocal(rr[64:65, :], pvx[64:65, :])
                    bp = ps.tile([64, 512], f32, tag="acc", bufs=2)
                    nc.tensor.matmul(bp[0:64, :], ones_t[64:65, 0:64], rr[64:65, :],
                                     start=True, stop=True)
                    bb = sb_l.tile([64, 512], f32r, tag="bb", bufs=2)
                    nc.vector.tensor_copy(bb[:], bp[0:64, :])
                    if par == 0:
                        nc.vector.tensor_mul(oT[0:64, c * 512:(c + 1) * 512],
                                             pvx[0:64, :], bb[:])
                    else:
                        tm = sb_l.tile([64, 512], f32r, tag="tm", bufs=1)
                        nc.vector.tensor_mul(tm[:], pvx[0:64, :], bb[:])
                        nc.sync.dma_start(out=oT[64:128, c * 512:(c + 1) * 512],
                                          in_=tm[:])
                # PE fillers: next projection groups / V tiles / output proj
                for fn in fills.get((hp, slot), []):
                    fn()

    nc.finalize()
    return nc


def _get_nc():
    global _NC_CACHE
    if _NC_CACHE is None:
        _NC_CACHE = _build_nc()
    return _NC_CACHE


def _slopes():
    start = 2.0 ** (-(2.0 ** (-(math.log2(H) - 3.0))))
    return np.array([start * start ** i for i in range(H)], dtype=np.float64)


def _host_prep(x, Wq, Aq, Bq, Wk, Ak, Bk, Wv, Av, Bv, Wp):
    f8 = np.float64
    weff = {}
    for nm, W, A, B in (("q", Wq, Aq, Bq), ("k", Wk, Ak, Bk), ("v", Wv, Av, Bv)):
        weff[nm] = (W.astype(f8) + 2.0 * (A.astype(f8) @ B.astype(f8)))
    weff["q"] = weff["q"] / math.sqrt(DH)          # fold 1/sqrt(dh) into q weights
    slopes = _slopes()

    pj = np.arange(128)
    pi = np.arange(128)
    trineg = np.where(pj[:, None] <= pi[None, :], 0.0, NEG).astype(np.float32)
    jj = np.arange(T, dtype=np.float64).reshape(16, 128).T   # [pj, jt] -> j

    in_maps = []
    for b in range(4):
        xT = np.ascontiguousarray(x[b].T)
        for hg in range(2):
            S = slice(hg * 512, hg * 512 + 512)
            # gv[pj, tt*8 + h] = exp(-(slope_h * j + C)), j = tt*128 + pj
            gv = np.stack([np.exp(-(slopes[hg * 8 + hl] * jj + CB))
                           for hl in range(HL)], axis=2)   # [128, 16, 8]
            gv = gv.reshape(128, 16 * HL).astype(np.float32)
            in_maps.append({
                "xT": xT,
                "wqT": np.ascontiguousarray(weff["q"][S].T).astype(np.float32),
                "wkT": np.ascontiguousarray(weff["k"][S].T).astype(np.float32),
                "wvT": np.ascontiguousarray(weff["v"][S].T).astype(np.float32),
                "wpT": np.ascontiguousarray(Wp[:, S].T),
                "ebias": gv,
                "trineg": trineg,
                "onesd": np.ones((128, 128), dtype=np.float32),
            })
    return in_maps


def run(inputs, trace=False):
    nc = _get_nc()
    inputs = {k: np.asarray(v, dtype=np.float32) for k, v in inputs.items()}
    in_maps = _host_prep(**inputs)
    res = run_bass_kernel_spmd(nc, in_maps, list(range(8)), trace=trace)
    outs = [np.asarray(res.results[i]["out"]) for i in range(8)]
    full = np.stack([outs[2 * b] + outs[2 * b + 1] for b in range(4)])
    return full.astype(np.float32), res


def kernel(**inputs):
    full, _ = run(inputs, trace=False)
    return full



# revision 39
# speedup vs baseline: 1.9081x; 1.9081x over previous
"""Multi-head attention (LoRA QKV + ALiBi + causal softmax + output proj) on 8 TRN2 cores.

Sharding: core = (batch b in 0..3, head-half in 0..1); each core handles one batch
element and 8 of the 16 heads.  LoRA is folded into effective weights on the host
(W_eff = W + 2*A@B, exact algebra).  Each core computes a partial projection output
(its 512 attention dims x full Wp rows); the host sums the two partials per batch.

ALiBi here ADDS slope*(i-j) (reference semantics), so every head attends to the
EARLIEST keys; key j's weight carries a factor exp(-slope*j).  Beyond
j > ~40/slope a key's relative contribution is < e^-21 -- the kernel folds
exp(-slope*j - C) into V on the host (gv), where it literally underflows to 0.0f
for steep heads.  So per head only the first m j-tiles (128 keys each) matter:

  m(head) = ceil(40 / (slope * 128)), capped at 16

Heads are re-paired by matching m and distributed so that both core-halves run the
same instruction stream with pair-slot profile SLOT_M = [16, 16, 5, 2] (j-tiles per
slot); the head->slot assignment differs per core only in the DATA (weight column
order, gv).  This cuts S/PV/exp work to ~67% and lets K-projection (only KCH[hp]
512-token chunks of kt are ever read) and V-projection (only active heads per
token-tile) shrink too.

On-core math (all matmuls in float32r = full PE speed for free-dim >= 256):
  qT[d,t] = sum_e wqT[e,d] * xT[e,t]          (wqT pre-scaled by 1/sqrt(dh) on host)
  kT[d,t], v[t,d] similar
  sT[j,i] = sum_d kT[d,j] qT[d,i]             (two heads packed per 64-row PE strip;
                                               the two 64-row matmuls run CONCURRENT
                                               via PE row-group tiling)
  p[j,i]  = exp(sT[j,i])                      (per-j factor exp(-slope*j-C) is in V)
  causal: p[j,i] = 0 where j > i              (gpsimd affine_select on diagonal tiles)
  pv[d,i] = sum_j v'[j,d] p[j,i]              (v' has a ones column -> row d=64 is the
                                               softmax denominator)
  outT[d,i] = pv[d,i] * recip(pv[64,i])       (approx-recip on DVE + ones-matmul bcast)
  out[t,e] = sum_d outT[d,t] * wpT[d,e]       (partial; host adds the other half)

Schedule: PE stream interleaves attention chunks with projection work (fills) so the
PE never waits on the ACT (exp) or DVE (normalize) chains; normalize has no PE
instruction before the fills, so chunk boundaries don't stall the PE p-state.
"""

import math
from contextlib import ExitStack

import numpy as np

import concourse.bacc as bacc
import concourse.mybir as mybir
import concourse.tile as tile
from concourse.bass_utils import run_bass_kernel_spmd

T, E, DH, H = 2048, 1024, 64, 16
HL = 8              # heads per core
NKT = 8             # contraction tiles of 128 over E
NTT = 16            # token tiles of 128 over T
CB = 12.0           # safety constant folded into gv

SLOT_M = [16, 16, 5, 2]       # j-tile cutoff per head-pair slot
KCH = [4, 4, 2, 1]            # kt 512-token chunks per slot = ceil(M/4)
# per-core head order (slot-major): chosen so each pair's true m fits its slot
HEADS_HALF = [
    [11, 12, 13, 14, 7, 6, 3, 2],
    [15, 10, 9, 8, 5, 4, 1, 0],
]


def _nact(tt):
    """Active head count at key-tile tt (heads whose slot still attends)."""
    return 2 * sum(1 for m in SLOT_M if m > tt)


_NC_CACHE = None


def _build_nc():
    f32 = mybir.dt.float32
    f32r = mybir.dt.float32r
    bf16 = mybir.dt.bfloat16
    Exp = mybir.ActivationFunctionType.Exp

    nc = bacc.Bacc(trn_type="TRN2", target_bir_lowering=False, debug=False)
    xT_d = nc.declare_dram_parameter("xT", [E, T], bf16, isOutput=False)
    wqT_d = nc.declare_dram_parameter("wqT", [E, 512], bf16, isOutput=False)
    wkT_d = nc.declare_dram_parameter("wkT", [E, 512], bf16, isOutput=False)
    wvT_d = nc.declare_dram_parameter("wvT", [E, 512], bf16, isOutput=False)
    wpT_d = nc.declare_dram_parameter("wpT", [512, E], bf16, isOutput=False)
    eb_d = nc.declare_dram_parameter("ebias", [128, 128], f32, isOutput=False)
    ones_d = nc.declare_dram_parameter("onesd", [128, 128], f32, isOutput=False)
    out_d = nc.declare_dram_parameter("out", [T, E], bf16, isOutput=True)

    with ExitStack() as st:
        tc = st.enter_context(tile.TileContext(nc))
        ps = st.enter_context(tc.tile_pool(name="ps", bufs=1, space="PSUM"))
        # psum tags: acc(2) + s(4) + pv(2) = 8 banks exactly
        sb_r = st.enter_context(tc.tile_pool(name="sbr", bufs=1, side="right"))
        sb_x = st.enter_context(tc.tile_pool(name="sbx", bufs=1, side="left"))
        sb_l = st.enter_context(tc.tile_pool(name="sbl", bufs=1, side="left"))

        # ---------- DMA plumbing ----------
        xts = []
        for k in range(NKT):
            xts.append(sb_x.tile([128, T], bf16, tag=f"xt{k}", bufs=1, name=f"xt{k}"))

        def dma_xt_chunk(ck):
            for k in range(NKT):
                nc.sync.dma_start(
                    out=xts[k][:, ck * 512:(ck + 1) * 512],
                    in_=xT_d[k * 128:(k + 1) * 128, ck * 512:(ck + 1) * 512])

        def dma_xt_half(ck, h):
            for k in range(NKT):
                o = ck * 512 + h * 256
                nc.sync.dma_start(out=xts[k][:, o:o + 256],
                                  in_=xT_d[k * 128:(k + 1) * 128, o:o + 256])

        dma_xt_half(0, 0)
        wvs = []
        for k in range(NKT):
            t = sb_l.tile([128, 512], bf16, tag="wst", bufs=8, name=f"wv{k}")
            nc.gpsimd.dma_start(out=t[:], in_=wvT_d[k * 128:(k + 1) * 128, :])
            wvs.append(t)
        gv_sb = sb_r.tile([128, 128], f32, tag="gv", bufs=1)
        ones_t = sb_r.tile([128, 64], f32r, tag="ones", bufs=1)

        qts = [None] * 4
        kts = [None] * 4
        wqk = [None] * 4
        outTs = [None] * 4

        def emit_wqk_dma(hp, queue=None):
            eng = queue or nc.gpsimd
            tiles = {}
            for which, wd in (("q", wqT_d), ("k", wkT_d)):
                wt = sb_l.tile([128, 1024], bf16, tag="wqk", bufs=2,
                               name=f"w{which}{hp}")
                src = wd[:, hp * 128:(hp + 1) * 128]
                src = src.rearrange("(k p) m -> p k m", p=128)
                eng.dma_start(out=wt.rearrange("p (k m) -> p k m", k=NKT), in_=src)
                tiles[which] = wt
            wqk[hp] = tiles
            qts[hp] = sb_l.tile([128, T], bf16, tag="qt", bufs=2, name=f"qt{hp}")
            kts[hp] = sb_l.tile([128, T], bf16, tag="kt", bufs=2, name=f"kt{hp}")

        def emit_qk_group(hp, which, tck):
            wt = wqk[hp][which]
            ot = qts[hp] if which == "q" else kts[hp]
            pq = ps.tile([128, 512], f32, tag="acc", bufs=2)
            for k in range(NKT):
                nc.tensor.matmul(pq[:], wt[:, k * 128:(k + 1) * 128],
                                 xts[k][:, tck * 512:(tck + 1) * 512],
                                 start=(k == 0), stop=(k == NKT - 1))
            nc.vector.tensor_copy(ot[:, tck * 512:(tck + 1) * 512], pq[:])

        vts = [None] * NTT

        def emit_v_group(tt):
            na = _nact(tt)           # active heads at this key tile (8, 6 or 4)
            pvm = ps.tile([128, 512], f32, tag="acc", bufs=2)
            for k in range(NKT):
                nc.tensor.matmul(pvm[:, 0:64 * na],
                                 xts[k][:, tt * 128:(tt + 1) * 128],
                                 wvs[k][:, 0:64 * na],
                                 start=(k == 0), stop=(k == NKT - 1))
            vt = sb_r.tile([128, na * 65], bf16, tag=f"v{tt}", bufs=1, name=f"v{tt}")
            v3 = vt.rearrange("p (h c) -> p h c", h=na)
            for h in range(na):
                nc.vector.tensor_scalar_mul(
                    v3[:, h, 0:64], pvm[:, h * 64:(h + 1) * 64],
                    gv_sb[:, tt * HL + h:tt * HL + h + 1])
            nc.vector.tensor_copy(
                v3[:, :, 64:65],
                gv_sb[:, tt * HL:tt * HL + na].rearrange("p (h c) -> p h c", c=1))
            vts[tt] = vt

        wps = [None] * 8

        def emit_wp_dma():
            for i in range(8):  # i = hp*2 + ec
                hp, ec = i // 2, i % 2
                t = sb_l.tile([128, 512], bf16, tag="wst", bufs=8, name=f"wp{i}")
                nc.gpsimd.dma_start(
                    out=t[:],
                    in_=wpT_d[hp * 128:(hp + 1) * 128,
                              ec * 512:(ec + 1) * 512])
                wps[i] = t

        def emit_proj_group(tt, ec):
            po = ps.tile([128, 512], f32, tag="acc", bufs=2)
            for hp in range(4):
                nc.tensor.matmul(po[:], outTs[hp][:, tt * 128:(tt + 1) * 128],
                                 wps[hp * 2 + ec][:], start=(hp == 0), stop=(hp == 3))
            ob = sb_l.tile([128, 512], bf16, tag="ob", bufs=2)
            nc.vector.tensor_copy(ob[:], po[:])
            nc.sync.dma_start(out=out_d[tt * 128:(tt + 1) * 128,
                                        ec * 512:(ec + 1) * 512],
                              in_=ob[:])

        # ---------- filler singles queue ----------
        # Fill work (QKV projections, V staging, output proj) is broken into
        # SINGLE-matmul thunks and pumped between attention tiles so the PE
        # never idles while ACT (exp) runs: the PE must stay near-100% busy in
        # every 3.4us HAM window or the clock drops to 1.2 GHz.
        # Queue is FIFO in deadline order; need_by = (slot_pos, c) in
        # processing order.  Group state (psum acc tile) lives in a closure:
        # acc tag bufs=2 and FIFO consumption mean at most 2 open groups.
        SLOT_ORDER = [3, 2, 0, 1]
        lateq = []   # items: (cost_ns, need_by, thunk, gid, is_last)
        _gid = [0]
        open_gid = [None]

        def q_push(cost, need_by, fn, gid=None, last=True):
            lateq.append((cost, need_by, fn, gid, last))

        def _pop_run():
            cost, _, fn, gid, last = lateq.pop(0)
            fn()
            open_gid[0] = None if (last or gid is None) else gid
            return cost

        def close_group():
            # finish the currently open psum-acc accumulation group so a
            # non-queue acc-tag alloc (normalize's bcast) can't deadlock
            while lateq and open_gid[0] is not None:
                _pop_run()

        def qk_singles(hp, which, tck, need_by):
            st = {}
            _gid[0] += 1
            g = _gid[0]

            def mk(k):
                def f():
                    if k == 0:
                        st["pq"] = ps.tile([128, 512], f32, tag="acc", bufs=2, name=f"pq{g}")
                    nc.tensor.matmul(st["pq"][:],
                                     wqk[hp][which][:, k * 128:(k + 1) * 128],
                                     xts[k][:, tck * 512:(tck + 1) * 512],
                                     start=(k == 0), stop=(k == NKT - 1))
                return f
            for k in range(NKT):
                q_push(213, need_by, mk(k), gid=g, last=False)

            def cast():
                ot = qts[hp] if which == "q" else kts[hp]
                nc.vector.tensor_copy(ot[:, tck * 512:(tck + 1) * 512],
                                      st["pq"][:])
            q_push(0, need_by, cast, gid=g, last=True)

        def qk_push(hp, tck, need_by):
            qk_singles(hp, "q", tck, need_by)
            if tck < KCH[hp]:
                qk_singles(hp, "k", tck, need_by)

        def v_singles(tt, need_by):
            na = _nact(tt)
            st = {}
            _gid[0] += 1
            g = _gid[0]

            def mk(k):
                def f():
                    if k == 0:
                        st["pvm"] = ps.tile([128, 512], f32, tag="acc", bufs=2, name=f"pvm{g}")
                    nc.tensor.matmul(st["pvm"][:, 0:64 * na],
                                     xts[k][:, tt * 128:(tt + 1) * 128],
                                     wvs[k][:, 0:64 * na],
                                     start=(k == 0), stop=(k == NKT - 1))
                return f
            for k in range(NKT):
                q_push(27 * na, need_by, mk(k), gid=g, last=False)

            def tailf():
                vt = sb_r.tile([128, na * 65], bf16, tag=f"v{tt}", bufs=1,
                               name=f"v{tt}")
                v3 = vt.rearrange("p (h c) -> p h c", h=na)
                for h in range(na):
                    nc.vector.tensor_scalar_mul(
                        v3[:, h, 0:64], st["pvm"][:, h * 64:(h + 1) * 64],
                        gv_sb[:, tt * HL + h:tt * HL + h + 1])
                nc.vector.tensor_copy(
                    v3[:, :, 64:65],
                    gv_sb[:, tt * HL:tt * HL + na].rearrange(
                        "p (h c) -> p h c", c=1))
                vts[tt] = vt
            q_push(0, need_by, tailf, gid=g, last=True)

        def proj_singles(tt, ec, need_by):
            st = {}
            _gid[0] += 1
            g = _gid[0]

            def mk(hp):
                def f():
                    if hp == 0:
                        st["po"] = ps.tile([128, 512], f32, tag="acc", bufs=2, name=f"po{g}")
                    nc.tensor.matmul(st["po"][:],
                                     outTs[hp][:, tt * 128:(tt + 1) * 128],
                                     wps[hp * 2 + ec][:],
                                     start=(hp == 0), stop=(hp == 3))
                return f
            for hp in range(4):
                q_push(213, need_by, mk(hp), gid=g, last=False)

            def tailf():
                ob = sb_l.tile([128, 512], bf16, tag="ob", bufs=2)
                nc.vector.tensor_copy(ob[:], st["po"][:])
                nc.sync.dma_start(out=out_d[tt * 128:(tt + 1) * 128,
                                            ec * 512:(ec + 1) * 512],
                                  in_=ob[:])
            q_push(0, need_by, tailf, gid=g, last=True)

        # enqueue everything in deadline order (constraints: wqk DMA only
        # after the previous slot's qk groups; wp DMA after the last V group)
        q_push(0, (0, 1), lambda: dma_xt_chunk(2))
        qk_push(3, 1, (0, 1))
        qk_push(3, 2, (0, 2))
        q_push(0, (0, 3), lambda: dma_xt_chunk(3))
        qk_push(3, 3, (0, 3))
        v_singles(2, (1, 0))
        v_singles(3, (1, 0))
        q_push(0, (1, 0), lambda: emit_wqk_dma(2))
        qk_push(2, 0, (1, 0))
        v_singles(4, (1, 1))
        qk_push(2, 1, (1, 1))
        qk_push(2, 2, (1, 2))
        qk_push(2, 3, (1, 3))
        q_push(0, (2, 0), lambda: emit_wqk_dma(0))
        qk_push(0, 0, (2, 0))
        for tt in (5, 6, 7):
            v_singles(tt, (2, 1))
        qk_push(0, 1, (2, 1))
        for tt in (8, 9, 10, 11):
            v_singles(tt, (2, 2))
        qk_push(0, 2, (2, 2))
        qk_push(0, 3, (2, 3))
        for tt in (12, 13, 14, 15):
            v_singles(tt, (2, 3))
        q_push(0, (3, 0), emit_wp_dma)
        q_push(0, (3, 0), lambda: emit_wqk_dma(1))
        qk_push(1, 0, (3, 0))
        qk_push(1, 1, (3, 1))
        qk_push(1, 2, (3, 2))
        qk_push(1, 3, (3, 3))

        debt = [0.0]

        def pump(ns):
            debt[0] = min(debt[0] + ns, 3000.0)
            while lateq and debt[0] >= lateq[0][0]:
                debt[0] -= _pop_run()

        def drain(upto):
            while lateq and lateq[0][1] <= upto:
                _pop_run()

        # ---------- preloop ----------
        nc.gpsimd.dma_start(out=gv_sb[:], in_=eb_d[:])
        nc.gpsimd.dma_start(out=ones_t[:], in_=ones_d[:, 0:64].bitcast(f32r))
        emit_wqk_dma(3)
        dma_xt_half(0, 1)
        dma_xt_chunk(1)
        for tt in range(2):
            emit_v_group(tt)
        emit_qk_group(3, "q", 0)
        emit_qk_group(3, "k", 0)

        # ---------- attention ----------
        # normalize: outT[d, i] = pv[d, i] * (1 / pv[64, i]).
        # approx recip needs a base-partition-0 AP (the custom DVE op
        # misreads offset APs); rows 0:64 are don't-care.
        def norm_dve_part(pv0, pv1):
            # reciprocal chain only (DVE): runs right at the pv stop so the
            # result is ready when the deferred PE part fires next chunk
            rrrs = []
            for pvx in (pv0, pv1):
                rr = sb_l.tile([65, 512], f32, tag="rr", bufs=1)
                nc.vector.reciprocal_approx_fast(rr[0:65, :], pvx[0:65, :])
                rrr = sb_l.tile([65, 512], f32r, tag="rrr", bufs=2)
                nc.vector.tensor_copy(rrr[64:65, :], rr[64:65, :])
                rrrs.append(rrr)
            return rrrs

        def norm_pe_part(oT, c, pv0, pv1, rrrs):
            for par, pvx in ((0, pv0), (1, pv1)):
                bp = ps.tile([64, 512], f32, tag="acc", bufs=2)
                nc.tensor.matmul(bp[0:64, :], ones_t[64:65, 0:64],
                                 rrrs[par][64:65, :], start=True, stop=True)
                bb = sb_l.tile([64, 512], bf16, tag="bb", bufs=2)
                nc.scalar.copy(bb[:], bp[0:64, :])
                if par == 0:
                    nc.vector.tensor_mul(oT[0:64, c * 512:(c + 1) * 512],
                                         pvx[0:64, :], bb[:])
                else:
                    tm = sb_l.tile([64, 512], bf16, tag="tm", bufs=1)
                    nc.vector.tensor_mul(tm[:], pvx[0:64, :], bb[:])
                    nc.sync.dma_start(out=oT[64:128, c * 512:(c + 1) * 512],
                                      in_=tm[:])

        pend = [None]

        for spos, hp in enumerate(SLOT_ORDER):
            drain((spos, 0))   # ensures this slot's wqk DMA (and tiles) exist
            qt, kt = qts[hp], kts[hp]
            oT = sb_r.tile([128, T], bf16, tag=f"ot{hp}", bufs=1, name=f"ot{hp}")
            outTs[hp] = oT
            for c in range(4):
                drain((spos, c))
                pv0 = ps.tile([128, 512], f32, tag="pv", bufs=2)
                pv1 = ps.tile([128, 512], f32, tag="pv", bufs=2)
                njt = min(4 * c + 4, SLOT_M[hp])
                h0off = 2 * hp * 65

                def emit_pv(jt, cw):
                    p01 = p01s[jt % 2]
                    nc.tensor.matmul(pv0[0:65, 512 - cw:512],
                                     vts[jt][:, h0off:h0off + 65],
                                     p01[:, 0:cw],
                                     start=(jt == 0), stop=(jt == njt - 1))
                    nc.tensor.matmul(pv1[0:65, 512 - cw:512],
                                     vts[jt][:, h0off + 65:h0off + 130],
                                     p01[:, 512:512 + cw],
                                     start=(jt == 0), stop=(jt == njt - 1))

                p01s = [None, None]
                cws = [0] * njt
                for jt in range(njt):
                    r = jt - 4 * c
                    # bf16 operands have no min-free-dim penalty: use the
                    # exact unmasked width per diagonal tile
                    cw = 512 - 128 * r if r > 0 else 512
                    mw = 128
                    ioff = c * 512 + (512 - cw)
                    cws[jt] = cw
                    s01 = ps.tile([128, 1024], f32, tag="s", bufs=2)
                    nc.tensor.matmul(s01[:, 0:cw], kt[0:64, jt * 128:(jt + 1) * 128],
                                     qt[0:64, ioff:ioff + cw], start=True, stop=True)
                    nc.tensor.matmul(s01[:, 512:512 + cw],
                                     kt[64:128, jt * 128:(jt + 1) * 128],
                                     qt[64:128, ioff:ioff + cw], start=True, stop=True)
                    p01 = sb_l.tile([128, 1024], bf16, tag="pt", bufs=2)
                    p01s[jt % 2] = p01
                    s3 = s01.rearrange("p (h m) -> p h m", h=2)
                    p3 = p01.rearrange("p (h m) -> p h m", h=2)
                    nc.scalar.activation(p3[:, :, 0:cw], s3[:, :, 0:cw], Exp)
                    if r >= 0:
                        # zero the j > i region at the head of the window:
                        # keep where (i - j) = (m - (mw - 128)) - pj >= 0
                        for off in (0, 512):
                            nc.gpsimd.affine_select(
                                out=p01[:, off:off + mw], in_=p01[:, off:off + mw],
                                compare_op=mybir.AluOpType.is_ge, fill=0.0,
                                base=-(mw - 128), pattern=[[1, mw]],
                                channel_multiplier=-1)
                    # software pipeline: PV runs one tile behind S, with
                    # pumped filler singles covering the exp latency.
                    # deficit per tile ~= ACT time - attention PE time (warm)
                    if jt > 0:
                        if (jt == min(2, njt - 1) and pend[0] is not None):
                            close_group()
                            p_hp, p_args = pend[0]
                            norm_pe_part(*p_args)
                            pend[0] = None
                            if p_hp == 1:   # unlocks proj for that chunk
                                for tt in range(4 * p_args[1],
                                                4 * p_args[1] + 4):
                                    for ec in range(2):
                                        proj_singles(tt, ec, (3, 9))
                        pump((172 + 2 * cw) / 1.2 - 1.25 * cw
                             + (250 if r >= 0 else 0))
                        emit_pv(jt - 1, cws[jt - 1])
                pump(600)
                emit_pv(njt - 1, cws[njt - 1])
                rrrs = norm_dve_part(pv0, pv1)
                pend[0] = (hp, (oT, c, pv0, pv1, rrrs))
        close_group()
        norm_pe_part(*pend[0][1])
        pend[0] = None
        for tt in range(12, 16):
            for ec in range(2):
                proj_singles(tt, ec, (3, 9))
        while lateq:
            _pop_run()

    nc.finalize()
    return nc


def _get_nc():
    global _NC_CACHE
    if _NC_CACHE is None:
        _NC_CACHE = _build_nc()
    return _NC_CACHE


def _slopes():
    start = 2.0 ** (-(2.0 ** (-(math.log2(H) - 3.0))))
    return np.array([start * start ** i for i in range(H)], dtype=np.float64)


def _host_prep(x, Wq, Aq, Bq, Wk, Ak, Bk, Wv, Av, Bv, Wp):
    f8 = np.float64
    weff = {}
    for nm, W, A, B in (("q", Wq, Aq, Bq), ("k", Wk, Ak, Bk), ("v", Wv, Av, Bv)):
        weff[nm] = (W.astype(f8) + 2.0 * (A.astype(f8) @ B.astype(f8)))
    weff["q"] = weff["q"] / math.sqrt(DH)          # fold 1/sqrt(dh) into q weights
    slopes = _slopes()

    jj = np.arange(T, dtype=np.float64).reshape(16, 128).T   # [pj, tt] -> j

    import ml_dtypes
    bf = ml_dtypes.bfloat16

    in_maps = []
    for b in range(4):
        xT = np.ascontiguousarray(x[b].T).astype(bf)
        for half in range(2):
            heads = HEADS_HALF[half]
            rows = np.concatenate([np.arange(h * 64, (h + 1) * 64) for h in heads])
            # gv[pj, tt*8 + hl] = exp(-(slope_h * j + C)), j = tt*128 + pj
            gv = np.stack([np.exp(-(slopes[heads[hl]] * jj + CB))
                           for hl in range(HL)], axis=2)   # [128, 16, 8]
            gv = gv.reshape(128, 16 * HL).astype(np.float32)
            in_maps.append({
                "xT": xT,
                "wqT": np.ascontiguousarray(weff["q"][rows].T).astype(bf),
                "wkT": np.ascontiguousarray(weff["k"][rows].T).astype(bf),
                "wvT": np.ascontiguousarray(weff["v"][rows].T).astype(bf),
                "wpT": np.ascontiguousarray(Wp[:, rows].T).astype(bf),
                "ebias": gv,
                "onesd": np.ones((128, 128), dtype=np.float32),
            })
    return in_maps


def run(inputs, trace=False):
    nc = _get_nc()
    inputs = {k: np.asarray(v, dtype=np.float32) for k, v in inputs.items()}
    in_maps = _host_prep(**inputs)
    res = run_bass_kernel_spmd(nc, in_maps, list(range(8)), trace=trace)
    outs = [np.asarray(res.results[i]["out"]).astype(np.float32)
            for i in range(8)]
    full = np.stack([outs[2 * b] + outs[2 * b + 1] for b in range(4)])
    return full.astype(np.float32), res


def kernel(**inputs):
    full, _ = run(inputs, trace=False)
    return full


# revision 40
# speedup vs baseline: 2.2577x; 1.1832x over previous
"""Multi-head attention (LoRA QKV + ALiBi + causal softmax + output proj) on 8 TRN2 cores.

Sharding: core = (batch b in 0..3, head-half in 0..1); each core handles one batch
element and 8 of the 16 heads.  LoRA is folded into effective weights on the host
(W_eff = W + 2*A@B, exact algebra).  Each core computes a partial projection output
(its 512 attention dims x full Wp rows); the host sums the two partials per batch.

ALiBi here ADDS slope*(i-j) (reference semantics), so every head attends to the
EARLIEST keys; key j's weight carries a factor exp(-slope*j).  Beyond
j > ~40/slope a key's relative contribution is < e^-21 -- the kernel folds
exp(-slope*j - C) into V on the host (gv), where it literally underflows to 0.0f
for steep heads.  So per head only the first m j-tiles (128 keys each) matter:

  m(head) = ceil(40 / (slope * 128)), capped at 16

Heads are re-paired by matching m and distributed so that both core-halves run the
same instruction stream with pair-slot profile SLOT_M = [16, 16, 5, 2] (j-tiles per
slot); the head->slot assignment differs per core only in the DATA (weight column
order, gv).  This cuts S/PV/exp work to ~67% and lets K-projection (only KCH[hp]
512-token chunks of kt are ever read) and V-projection (only active heads per
token-tile) shrink too.

On-core math (x / weights / p / v' / outT in bf16, psum + softmax chain f32):
  qT[d,t] = sum_e wqT[e,d] * xT[e,t]          (wqT pre-scaled by 1/sqrt(dh) on host)
  kT[d,t], v[t,d] similar
  sT[j,i] = sum_d kT[d,j] qT[d,i]             (two heads packed per 64-row PE strip;
                                               the two 64-row matmuls run CONCURRENT
                                               via PE row-group tiling)
  p[j,i]  = exp(sT[j,i])                      (per-j factor exp(-slope*j-C) is in V)
  causal: p[j,i] = 0 where j > i              (gpsimd affine_select on diagonal tiles)
  pv[d,i] = sum_j v'[j,d] p[j,i]              (v' has a ones column -> row d=64 is the
                                               softmax denominator)
  outT[d,i] = pv[d,i] * recip(pv[64,i])       (approx-recip on DVE + ones-matmul bcast)
  out[t,e] = sum_d outT[d,t] * wpT[d,e]       (partial; host adds the other half)

Schedule: the PE must stay near-100% busy in every ~3.4us window or the HAM clock
gate drops it from 2.4 to 1.2 GHz, so ALL filler work (QKV projections, V staging,
output projection) is broken into single-matmul thunks in a deadline-ordered FIFO
(lateq) and pumped between attention tiles; PV runs one tile behind S so exp's
latency is hidden; each chunk's normalize splits into an immediate DVE part
(reciprocal) and a PE part (broadcast matmul + muls) deferred into the next chunk.
"""

import math
from contextlib import ExitStack

import numpy as np

import concourse.bacc as bacc
import concourse.mybir as mybir
import concourse.tile as tile
from concourse.bass_utils import run_bass_kernel_spmd

T, E, DH, H = 2048, 1024, 64, 16
HL = 8              # heads per core
NKT = 8             # contraction tiles of 128 over E
NTT = 16            # token tiles of 128 over T
CB = 12.0           # safety constant folded into gv

SLOT_M = [16, 16, 5, 2]       # j-tile cutoff per head-pair slot
KCH = [4, 4, 2, 1]            # kt 512-token chunks per slot = ceil(M/4)
# per-core head order (slot-major): chosen so each pair's true m fits its slot
HEADS_HALF = [
    [11, 12, 13, 14, 7, 6, 3, 2],
    [15, 10, 9, 8, 5, 4, 1, 0],
]


def _nact(tt):
    """Active head count at key-tile tt (heads whose slot still attends)."""
    return 2 * sum(1 for m in SLOT_M if m > tt)


_NC_CACHE = None


def _build_nc():
    f32 = mybir.dt.float32
    f32r = mybir.dt.float32r
    bf16 = mybir.dt.bfloat16
    Exp = mybir.ActivationFunctionType.Exp

    nc = bacc.Bacc(trn_type="TRN2", target_bir_lowering=False, debug=False)
    xT_d = nc.declare_dram_parameter("xT", [E, T], bf16, isOutput=False)
    wqT_d = nc.declare_dram_parameter("wqT", [E, 512], bf16, isOutput=False)
    wkT_d = nc.declare_dram_parameter("wkT", [E, 512], bf16, isOutput=False)
    wvT_d = nc.declare_dram_parameter("wvT", [E, 512], bf16, isOutput=False)
    wpT_d = nc.declare_dram_parameter("wpT", [512, E], bf16, isOutput=False)
    eb_d = nc.declare_dram_parameter("ebias", [128, 128], f32, isOutput=False)
    ones_d = nc.declare_dram_parameter("onesd", [128, 128], f32, isOutput=False)
    out_d = nc.declare_dram_parameter("out", [T, E], bf16, isOutput=True)

    with ExitStack() as st:
        tc = st.enter_context(tile.TileContext(nc))
        ps = st.enter_context(tc.tile_pool(name="ps", bufs=1, space="PSUM"))
        # psum tags: acc(2) + s(4) + pv(2) = 8 banks exactly
        sb_r = st.enter_context(tc.tile_pool(name="sbr", bufs=1, side="right"))
        sb_x = st.enter_context(tc.tile_pool(name="sbx", bufs=1, side="left"))
        sb_l = st.enter_context(tc.tile_pool(name="sbl", bufs=1, side="left"))

        # ---------- DMA plumbing ----------
        xts = []
        for k in range(NKT):
            xts.append(sb_x.tile([128, T], bf16, tag=f"xt{k}", bufs=1, name=f"xt{k}"))

        def dma_xt_chunk(ck):
            for k in range(NKT):
                nc.sync.dma_start(
                    out=xts[k][:, ck * 512:(ck + 1) * 512],
                    in_=xT_d[k * 128:(k + 1) * 128, ck * 512:(ck + 1) * 512])

        def dma_xt_half(ck, h):
            for k in range(NKT):
                o = ck * 512 + h * 256
                nc.sync.dma_start(out=xts[k][:, o:o + 256],
                                  in_=xT_d[k * 128:(k + 1) * 128, o:o + 256])

        dma_xt_half(0, 0)
        wvs = []
        for k in range(NKT):
            t = sb_l.tile([128, 512], bf16, tag="wst", bufs=8, name=f"wv{k}")
            nc.gpsimd.dma_start(out=t[:], in_=wvT_d[k * 128:(k + 1) * 128, :])
            wvs.append(t)
        gv_sb = sb_r.tile([128, 128], f32, tag="gv", bufs=1)
        ones_t = sb_r.tile([128, 64], f32r, tag="ones", bufs=1)

        qts = [None] * 4
        kts = [None] * 4
        wqk = [None] * 4
        outTs = [None] * 4

        def emit_wqk_dma(hp, queue=None):
            eng = queue or nc.gpsimd
            tiles = {}
            for which, wd in (("q", wqT_d), ("k", wkT_d)):
                wt = sb_l.tile([128, 1024], bf16, tag="wqk", bufs=2,
                               name=f"w{which}{hp}")
                src = wd[:, hp * 128:(hp + 1) * 128]
                src = src.rearrange("(k p) m -> p k m", p=128)
                eng.dma_start(out=wt.rearrange("p (k m) -> p k m", k=NKT), in_=src)
                tiles[which] = wt
            wqk[hp] = tiles
            qts[hp] = sb_l.tile([128, T], bf16, tag="qt", bufs=2, name=f"qt{hp}")
            kts[hp] = sb_l.tile([128, T], bf16, tag="kt", bufs=2, name=f"kt{hp}")

        def emit_qk_group(hp, which, tck):
            wt = wqk[hp][which]
            ot = qts[hp] if which == "q" else kts[hp]
            pq = ps.tile([128, 512], f32, tag="acc", bufs=2)
            for k in range(NKT):
                nc.tensor.matmul(pq[:], wt[:, k * 128:(k + 1) * 128],
                                 xts[k][:, tck * 512:(tck + 1) * 512],
                                 start=(k == 0), stop=(k == NKT - 1))
            nc.vector.tensor_copy(ot[:, tck * 512:(tck + 1) * 512], pq[:])

        vts = [None] * NTT

        def emit_v_group(tt):
            na = _nact(tt)           # active heads at this key tile (8, 6 or 4)
            pvm = ps.tile([128, 512], f32, tag="acc", bufs=2)
            for k in range(NKT):
                nc.tensor.matmul(pvm[:, 0:64 * na],
                                 xts[k][:, tt * 128:(tt + 1) * 128],
                                 wvs[k][:, 0:64 * na],
                                 start=(k == 0), stop=(k == NKT - 1))
            vt = sb_r.tile([128, na * 65], bf16, tag=f"v{tt}", bufs=1, name=f"v{tt}")
            v3 = vt.rearrange("p (h c) -> p h c", h=na)
            for h in range(na):
                nc.vector.tensor_scalar_mul(
                    v3[:, h, 0:64], pvm[:, h * 64:(h + 1) * 64],
                    gv_sb[:, tt * HL + h:tt * HL + h + 1])
            nc.vector.tensor_copy(
                v3[:, :, 64:65],
                gv_sb[:, tt * HL:tt * HL + na].rearrange("p (h c) -> p h c", c=1))
            vts[tt] = vt

        wps = [None] * 8

        def emit_wp_dma():
            for i in range(8):  # i = hp*2 + ec
                hp, ec = i // 2, i % 2
                t = sb_l.tile([128, 512], bf16, tag="wst", bufs=8, name=f"wp{i}")
                nc.gpsimd.dma_start(
                    out=t[:],
                    in_=wpT_d[hp * 128:(hp + 1) * 128,
                              ec * 512:(ec + 1) * 512])
                wps[i] = t

        def emit_proj_group(tt, ec):
            po = ps.tile([128, 512], f32, tag="acc", bufs=2)
            for hp in range(4):
                nc.tensor.matmul(po[:], outTs[hp][:, tt * 128:(tt + 1) * 128],
                                 wps[hp * 2 + ec][:], start=(hp == 0), stop=(hp == 3))
            ob = sb_l.tile([128, 512], bf16, tag="ob", bufs=2)
            nc.vector.tensor_copy(ob[:], po[:])
            nc.sync.dma_start(out=out_d[tt * 128:(tt + 1) * 128,
                                        ec * 512:(ec + 1) * 512],
                              in_=ob[:])

        # ---------- filler singles queue ----------
        # Fill work (QKV projections, V staging, output proj) is broken into
        # SINGLE-matmul thunks and pumped between attention tiles so the PE
        # never idles while ACT (exp) runs: the PE must stay near-100% busy in
        # every 3.4us HAM window or the clock drops to 1.2 GHz.
        # Queue is FIFO in deadline order; need_by = (slot_pos, c) in
        # processing order.  Group state (psum acc tile) lives in a closure:
        # acc tag bufs=2 and FIFO consumption mean at most 2 open groups.
        SLOT_ORDER = [3, 2, 0, 1]
        lateq = []   # items: (cost_ns, need_by, thunk, gid, is_last)
        _gid = [0]
        open_gid = [None]

        def q_push(cost, need_by, fn, gid=None, last=True):
            lateq.append((cost, need_by, fn, gid, last))

        def _pop_run():
            cost, _, fn, gid, last = lateq.pop(0)
            fn()
            open_gid[0] = None if (last or gid is None) else gid
            return cost

        def close_group():
            # finish the currently open psum-acc accumulation group so a
            # non-queue acc-tag alloc (normalize's bcast) can't deadlock
            while lateq and open_gid[0] is not None:
                _pop_run()

        def qk_singles(hp, which, tck, need_by):
            st = {}
            _gid[0] += 1
            g = _gid[0]

            def mk(k):
                def f():
                    if k == 0:
                        st["pq"] = ps.tile([128, 512], f32, tag="acc", bufs=2, name=f"pq{g}")
                    nc.tensor.matmul(st["pq"][:],
                                     wqk[hp][which][:, k * 128:(k + 1) * 128],
                                     xts[k][:, tck * 512:(tck + 1) * 512],
                                     start=(k == 0), stop=(k == NKT - 1))
                return f
            for k in range(NKT):
                q_push(213, need_by, mk(k), gid=g, last=False)

            def cast():
                ot = qts[hp] if which == "q" else kts[hp]
                nc.vector.tensor_copy(ot[:, tck * 512:(tck + 1) * 512],
                                      st["pq"][:])
            q_push(0, need_by, cast, gid=g, last=True)

        def qk_push(hp, tck, need_by):
            qk_singles(hp, "q", tck, need_by)
            if tck < KCH[hp]:
                qk_singles(hp, "k", tck, need_by)

        def v_singles(tt, need_by):
            na = _nact(tt)
            st = {}
            _gid[0] += 1
            g = _gid[0]

            def mk(k):
                def f():
                    if k == 0:
                        st["pvm"] = ps.tile([128, 512], f32, tag="acc", bufs=2, name=f"pvm{g}")
                    nc.tensor.matmul(st["pvm"][:, 0:64 * na],
                                     xts[k][:, tt * 128:(tt + 1) * 128],
                                     wvs[k][:, 0:64 * na],
                                     start=(k == 0), stop=(k == NKT - 1))
                return f
            for k in range(NKT):
                q_push(27 * na, need_by, mk(k), gid=g, last=False)

            def tailf():
                vt = sb_r.tile([128, na * 65], bf16, tag=f"v{tt}", bufs=1,
                               name=f"v{tt}")
                v3 = vt.rearrange("p (h c) -> p h c", h=na)
                for h in range(na):
                    nc.vector.tensor_scalar_mul(
                        v3[:, h, 0:64], st["pvm"][:, h * 64:(h + 1) * 64],
                        gv_sb[:, tt * HL + h:tt * HL + h + 1])
                nc.vector.tensor_copy(
                    v3[:, :, 64:65],
                    gv_sb[:, tt * HL:tt * HL + na].rearrange(
                        "p (h c) -> p h c", c=1))
                vts[tt] = vt
            q_push(0, need_by, tailf, gid=g, last=True)

        def proj_singles(tt, ec, need_by):
            st = {}
            _gid[0] += 1
            g = _gid[0]

            def mk(hp):
                def f():
                    if hp == 0:
                        st["po"] = ps.tile([128, 512], f32, tag="acc", bufs=2, name=f"po{g}")
                    nc.tensor.matmul(st["po"][:],
                                     outTs[hp][:, tt * 128:(tt + 1) * 128],
                                     wps[hp * 2 + ec][:],
                                     start=(hp == 0), stop=(hp == 3))
                return f
            for hp in range(4):
                q_push(213, need_by, mk(hp), gid=g, last=False)

            def tailf():
                ob = sb_l.tile([128, 512], bf16, tag="ob", bufs=2)
                nc.vector.tensor_copy(ob[:], st["po"][:])
                nc.sync.dma_start(out=out_d[tt * 128:(tt + 1) * 128,
                                            ec * 512:(ec + 1) * 512],
                                  in_=ob[:])
            q_push(0, need_by, tailf, gid=g, last=True)

        # enqueue everything in deadline order (constraints: wqk DMA only
        # after the previous slot's qk groups; wp DMA after the last V group)
        q_push(0, (0, 1), lambda: dma_xt_chunk(2))
        qk_push(3, 1, (0, 1))
        qk_push(3, 2, (0, 2))
        q_push(0, (0, 3), lambda: dma_xt_chunk(3))
        qk_push(3, 3, (0, 3))
        v_singles(2, (1, 0))
        v_singles(3, (1, 0))
        q_push(0, (1, 0), lambda: emit_wqk_dma(2))
        qk_push(2, 0, (1, 0))
        v_singles(4, (1, 1))
        qk_push(2, 1, (1, 1))
        qk_push(2, 2, (1, 2))
        qk_push(2, 3, (1, 3))
        q_push(0, (2, 0), lambda: emit_wqk_dma(0))
        qk_push(0, 0, (2, 0))
        for tt in (5, 6, 7):
            v_singles(tt, (2, 1))
        qk_push(0, 1, (2, 1))
        for tt in (8, 9, 10, 11):
            v_singles(tt, (2, 2))
        qk_push(0, 2, (2, 2))
        qk_push(0, 3, (2, 3))
        for tt in (12, 13, 14, 15):
            v_singles(tt, (2, 3))
        q_push(0, (3, 0), emit_wp_dma)
        q_push(0, (3, 0), lambda: emit_wqk_dma(1))
        qk_push(1, 0, (3, 0))
        qk_push(1, 1, (3, 1))
        qk_push(1, 2, (3, 2))
        qk_push(1, 3, (3, 3))

        debt = [0.0]

        def pump(ns):
            debt[0] = min(debt[0] + ns, 3000.0)
            while lateq and debt[0] >= lateq[0][0]:
                debt[0] -= _pop_run()

        def drain(upto):
            while lateq and lateq[0][1] <= upto:
                _pop_run()

        # ---------- preloop ----------
        nc.gpsimd.dma_start(out=gv_sb[:], in_=eb_d[:])
        nc.gpsimd.dma_start(out=ones_t[:], in_=ones_d[:, 0:64].bitcast(f32r))
        emit_wqk_dma(3)
        dma_xt_half(0, 1)
        dma_xt_chunk(1)
        for tt in range(2):
            emit_v_group(tt)
        emit_qk_group(3, "q", 0)
        emit_qk_group(3, "k", 0)

        # ---------- attention ----------
        # normalize: outT[d, i] = pv[d, i] * (1 / pv[64, i]).
        # approx recip needs a base-partition-0 AP (the custom DVE op
        # misreads offset APs); rows 0:64 are don't-care.
        def norm_dve_part(pv0, pv1):
            # reciprocal chain only (DVE): runs right at the pv stop so the
            # result is ready when the deferred PE part fires next chunk
            rrrs = []
            for pvx in (pv0, pv1):
                rr = sb_l.tile([65, 512], f32, tag="rr", bufs=1)
                nc.vector.reciprocal_approx_fast(rr[0:65, :], pvx[0:65, :])
                rrr = sb_l.tile([65, 512], f32r, tag="rrr", bufs=2)
                nc.vector.tensor_copy(rrr[64:65, :], rr[64:65, :])
                rrrs.append(rrr)
            return rrrs

        def norm_pe_part(oT, c, pv0, pv1, rrrs):
            for par, pvx in ((0, pv0), (1, pv1)):
                bp = ps.tile([64, 512], f32, tag="acc", bufs=2)
                nc.tensor.matmul(bp[0:64, :], ones_t[64:65, 0:64],
                                 rrrs[par][64:65, :], start=True, stop=True)
                bb = sb_l.tile([64, 512], bf16, tag="bb", bufs=2)
                nc.scalar.copy(bb[:], bp[0:64, :])
                if par == 0:
                    nc.vector.tensor_mul(oT[0:64, c * 512:(c + 1) * 512],
                                         pvx[0:64, :], bb[:])
                else:
                    tm = sb_l.tile([64, 512], bf16, tag="tm", bufs=1)
                    nc.vector.tensor_mul(tm[:], pvx[0:64, :], bb[:])
                    nc.sync.dma_start(out=oT[64:128, c * 512:(c + 1) * 512],
                                      in_=tm[:])

        pend = [None]

        for spos, hp in enumerate(SLOT_ORDER):
            drain((spos, 0))   # ensures this slot's wqk DMA (and tiles) exist
            qt, kt = qts[hp], kts[hp]
            oT = sb_r.tile([128, T], bf16, tag=f"ot{hp}", bufs=1, name=f"ot{hp}")
            outTs[hp] = oT
            for c in range(4):
                drain((spos, c))
                pv0 = ps.tile([128, 512], f32, tag="pv", bufs=2)
                pv1 = ps.tile([128, 512], f32, tag="pv", bufs=2)
                njt = min(4 * c + 4, SLOT_M[hp])
                h0off = 2 * hp * 65

                def emit_pv(jt, cw):
                    p01 = p01s[jt % 2]
                    nc.tensor.matmul(pv0[0:65, 512 - cw:512],
                                     vts[jt][:, h0off:h0off + 65],
                                     p01[:, 0:cw],
                                     start=(jt == 0), stop=(jt == njt - 1))
                    nc.tensor.matmul(pv1[0:65, 512 - cw:512],
                                     vts[jt][:, h0off + 65:h0off + 130],
                                     p01[:, 512:512 + cw],
                                     start=(jt == 0), stop=(jt == njt - 1))

                p01s = [None, None]
                cws = [0] * njt
                for jt in range(njt):
                    r = jt - 4 * c
                    # bf16 operands have no min-free-dim penalty: use the
                    # exact unmasked width per diagonal tile
                    cw = 512 - 128 * r if r > 0 else 512
                    mw = 128
                    ioff = c * 512 + (512 - cw)
                    cws[jt] = cw
                    s01 = ps.tile([128, 1024], f32, tag="s", bufs=2)
                    nc.tensor.matmul(s01[:, 0:cw], kt[0:64, jt * 128:(jt + 1) * 128],
                                     qt[0:64, ioff:ioff + cw], start=True, stop=True)
                    nc.tensor.matmul(s01[:, 512:512 + cw],
                                     kt[64:128, jt * 128:(jt + 1) * 128],
                                     qt[64:128, ioff:ioff + cw], start=True, stop=True)
                    p01 = sb_l.tile([128, 1024], bf16, tag="pt", bufs=2)
                    p01s[jt % 2] = p01
                    s3 = s01.rearrange("p (h m) -> p h m", h=2)
                    p3 = p01.rearrange("p (h m) -> p h m", h=2)
                    nc.scalar.activation(p3[:, :, 0:cw], s3[:, :, 0:cw], Exp)
                    if r >= 0:
                        # zero the j > i region at the head of the window:
                        # keep where (i - j) = (m - (mw - 128)) - pj >= 0
                        for off in (0, 512):
                            nc.gpsimd.affine_select(
                                out=p01[:, off:off + mw], in_=p01[:, off:off + mw],
                                compare_op=mybir.AluOpType.is_ge, fill=0.0,
                                base=-(mw - 128), pattern=[[1, mw]],
                                channel_multiplier=-1)
                    # software pipeline: PV runs one tile behind S, with
                    # pumped filler singles covering the exp latency.
                    # deficit per tile ~= ACT time - attention PE time (warm)
                    if jt > 0:
                        if (jt == min(2, njt - 1) and pend[0] is not None):
                            close_group()
                            p_hp, p_args = pend[0]
                            norm_pe_part(*p_args)
                            pend[0] = None
                            if p_hp == 1:   # unlocks proj for that chunk
                                for tt in range(4 * p_args[1],
                                                4 * p_args[1] + 4):
                                    for ec in range(2):
                                        proj_singles(tt, ec, (3, 9))
                        pump((172 + 2 * cw) / 1.2 - 1.25 * cw
                             + (250 if r >= 0 else 0))
                        emit_pv(jt - 1, cws[jt - 1])
                pump(600)
                emit_pv(njt - 1, cws[njt - 1])
                rrrs = norm_dve_part(pv0, pv1)
                pend[0] = (hp, (oT, c, pv0, pv1, rrrs))
        close_group()
        norm_pe_part(*pend[0][1])
        pend[0] = None
        for tt in range(12, 16):
            for ec in range(2):
                proj_singles(tt, ec, (3, 9))
        while lateq:
            _pop_run()

    nc.finalize()
    return nc


def _get_nc():
    global _NC_CACHE
    if _NC_CACHE is None:
        _NC_CACHE = _build_nc()
    return _NC_CACHE


def _slopes():
    start = 2.0 ** (-(2.0 ** (-(math.log2(H) - 3.0))))
    return np.array([start * start ** i for i in range(H)], dtype=np.float64)


def _host_prep(x, Wq, Aq, Bq, Wk, Ak, Bk, Wv, Av, Bv, Wp):
    f8 = np.float64
    weff = {}
    for nm, W, A, B in (("q", Wq, Aq, Bq), ("k", Wk, Ak, Bk), ("v", Wv, Av, Bv)):
        weff[nm] = (W.astype(f8) + 2.0 * (A.astype(f8) @ B.astype(f8)))
    weff["q"] = weff["q"] / math.sqrt(DH)          # fold 1/sqrt(dh) into q weights
    slopes = _slopes()

    jj = np.arange(T, dtype=np.float64).reshape(16, 128).T   # [pj, tt] -> j

    import ml_dtypes
    bf = ml_dtypes.bfloat16

    in_maps = []
    for b in range(4):
        xT = np.ascontiguousarray(x[b].T).astype(bf)
        for half in range(2):
            heads = HEADS_HALF[half]
            rows = np.concatenate([np.arange(h * 64, (h + 1) * 64) for h in heads])
            # gv[pj, tt*8 + hl] = exp(-(slope_h * j + C)), j = tt*128 + pj
            gv = np.stack([np.exp(-(slopes[heads[hl]] * jj + CB))
                           for hl in range(HL)], axis=2)   # [128, 16, 8]
            gv = gv.reshape(128, 16 * HL).astype(np.float32)
            in_maps.append({
                "xT": xT,
                "wqT": np.ascontiguousarray(weff["q"][rows].T).astype(bf),
                "wkT": np.ascontiguousarray(weff["k"][rows].T).astype(bf),
                "wvT": np.ascontiguousarray(weff["v"][rows].T).astype(bf),
                "wpT": np.ascontiguousarray(Wp[:, rows].T).astype(bf),
                "ebias": gv,
                "onesd": np.ones((128, 128), dtype=np.float32),
            })
    return in_maps


def run(inputs, trace=False):
    nc = _get_nc()
    inputs = {k: np.asarray(v, dtype=np.float32) for k, v in inputs.items()}
    in_maps = _host_prep(**inputs)
    res = run_bass_kernel_spmd(nc, in_maps, list(range(8)), trace=trace)
    outs = [np.asarray(res.results[i]["out"]).astype(np.float32)
            for i in range(8)]
    full = np.stack([outs[2 * b] + outs[2 * b + 1] for b in range(4)])
    return full.astype(np.float32), res


def kernel(**inputs):
    full, _ = run(inputs, trace=False)
    return full


# revision 42
# speedup vs baseline: 2.2651x; 1.0033x over previous
"""Multi-head attention (LoRA QKV + ALiBi + causal softmax + output proj) on 8 TRN2 cores.

Sharding: core = (batch b in 0..3, head-half in 0..1); each core handles one batch
element and 8 of the 16 heads.  LoRA is folded into effective weights on the host
(W_eff = W + 2*A@B, exact algebra).  Each core computes a partial projection output
(its 512 attention dims x full Wp rows); the host sums the two partials per batch.

ALiBi here ADDS slope*(i-j) (reference semantics), so every head attends to the
EARLIEST keys; key j's weight carries a factor exp(-slope*j).  Beyond
j > ~40/slope a key's relative contribution is < e^-21 -- the kernel folds
exp(-slope*j - C) into V on the host (gv), where it literally underflows to 0.0f
for steep heads.  So per head only the first m j-tiles (128 keys each) matter:

  m(head) = ceil(40 / (slope * 128)), capped at 16

Heads are re-paired by matching m and distributed so that both core-halves run the
same instruction stream with pair-slot profile SLOT_M = [16, 16, 5, 2] (j-tiles per
slot); the head->slot assignment differs per core only in the DATA (weight column
order, gv).  This cuts S/PV/exp work to ~67% and lets K-projection (only KCH[hp]
512-token chunks of kt are ever read) and V-projection (only active heads per
token-tile) shrink too.

On-core math (x / weights / p / v' / outT in bf16, psum + softmax chain f32):
  qT[d,t] = sum_e wqT[e,d] * xT[e,t]          (wqT pre-scaled by 1/sqrt(dh) on host)
  kT[d,t], v[t,d] similar
  sT[j,i] = sum_d kT[d,j] qT[d,i]             (two heads packed per 64-row PE strip;
                                               the two 64-row matmuls run CONCURRENT
                                               via PE row-group tiling)
  p[j,i]  = exp(sT[j,i])                      (per-j factor exp(-slope*j-C) is in V)
  causal: p[j,i] = 0 where j > i              (gpsimd affine_select on diagonal tiles)
  pv[d,i] = sum_j v'[j,d] p[j,i]              (v' has a ones column -> row d=64 is the
                                               softmax denominator)
  outT[d,i] = pv[d,i] * recip(pv[64,i])       (approx-recip on DVE + ones-matmul bcast)
  out[t,e] = sum_d outT[d,t] * wpT[d,e]       (partial; host adds the other half)

Schedule: the PE must stay near-100% busy in every ~3.4us window or the HAM clock
gate drops it from 2.4 to 1.2 GHz, so ALL filler work (QKV projections, V staging,
output projection) is broken into single-matmul thunks in a deadline-ordered FIFO
(lateq) and pumped between attention tiles; PV runs one tile behind S so exp's
latency is hidden; each chunk's normalize splits into an immediate DVE part
(reciprocal) and a PE part (broadcast matmul + muls) deferred into the next chunk.
"""

import math
from contextlib import ExitStack

import numpy as np

import concourse.bacc as bacc
import concourse.mybir as mybir
import concourse.tile as tile
from concourse.bass_utils import run_bass_kernel_spmd

T, E, DH, H = 2048, 1024, 64, 16
HL = 8              # heads per core
NKT = 8             # contraction tiles of 128 over E
NTT = 16            # token tiles of 128 over T
CB = 12.0           # safety constant folded into gv

SLOT_M = [16, 16, 5, 2]       # j-tile cutoff per head-pair slot
KCH = [4, 4, 2, 1]            # kt 512-token chunks per slot = ceil(M/4)
# per-core head order (slot-major): chosen so each pair's true m fits its slot
HEADS_HALF = [
    [11, 12, 13, 14, 7, 6, 3, 2],
    [15, 10, 9, 8, 5, 4, 1, 0],
]


def _nact(tt):
    """Active head count at key-tile tt (heads whose slot still attends)."""
    return 2 * sum(1 for m in SLOT_M if m > tt)


_NC_CACHE = None


def _build_nc():
    f32 = mybir.dt.float32
    f32r = mybir.dt.float32r
    bf16 = mybir.dt.bfloat16
    Exp = mybir.ActivationFunctionType.Exp

    nc = bacc.Bacc(trn_type="TRN2", target_bir_lowering=False, debug=False)
    xT_d = nc.declare_dram_parameter("xT", [E, T], bf16, isOutput=False)
    wqT_d = nc.declare_dram_parameter("wqT", [E, 512], bf16, isOutput=False)
    wkT_d = nc.declare_dram_parameter("wkT", [E, 512], bf16, isOutput=False)
    wvT_d = nc.declare_dram_parameter("wvT", [E, 512], bf16, isOutput=False)
    wpT_d = nc.declare_dram_parameter("wpT", [512, E], bf16, isOutput=False)
    eb_d = nc.declare_dram_parameter("ebias", [128, 128], f32, isOutput=False)
    ones_d = nc.declare_dram_parameter("onesd", [128, 128], f32, isOutput=False)
    out_d = nc.declare_dram_parameter("out", [T, E], bf16, isOutput=True)

    with ExitStack() as st:
        tc = st.enter_context(tile.TileContext(nc))
        ps = st.enter_context(tc.tile_pool(name="ps", bufs=1, space="PSUM"))
        # psum tags: acc(2) + s(4) + pv(2) = 8 banks exactly
        sb_r = st.enter_context(tc.tile_pool(name="sbr", bufs=1, side="right"))
        sb_x = st.enter_context(tc.tile_pool(name="sbx", bufs=1, side="left"))
        sb_l = st.enter_context(tc.tile_pool(name="sbl", bufs=1, side="left"))

        # ---------- DMA plumbing ----------
        xts = []
        for k in range(NKT):
            xts.append(sb_x.tile([128, T], bf16, tag=f"xt{k}", bufs=1, name=f"xt{k}"))

        def dma_xt_chunk(ck):
            for k in range(NKT):
                nc.sync.dma_start(
                    out=xts[k][:, ck * 512:(ck + 1) * 512],
                    in_=xT_d[k * 128:(k + 1) * 128, ck * 512:(ck + 1) * 512])

        def dma_xt_half(ck, h):
            for k in range(NKT):
                o = ck * 512 + h * 256
                nc.sync.dma_start(out=xts[k][:, o:o + 256],
                                  in_=xT_d[k * 128:(k + 1) * 128, o:o + 256])

        dma_xt_half(0, 0)
        wvs = []
        for k in range(NKT):
            t = sb_l.tile([128, 512], bf16, tag="wst", bufs=8, name=f"wv{k}")
            nc.gpsimd.dma_start(out=t[:], in_=wvT_d[k * 128:(k + 1) * 128, :])
            wvs.append(t)
        gv_sb = sb_r.tile([128, 128], f32, tag="gv", bufs=1)
        ones_t = sb_r.tile([128, 64], f32r, tag="ones", bufs=1)

        qts = [None] * 4
        kts = [None] * 4
        wqk = [None] * 4
        outTs = [None] * 4

        def emit_wqk_dma(hp, queue=None):
            eng = queue or nc.gpsimd
            tiles = {}
            for which, wd in (("q", wqT_d), ("k", wkT_d)):
                wt = sb_l.tile([128, 1024], bf16, tag="wqk", bufs=2,
                               name=f"w{which}{hp}")
                src = wd[:, hp * 128:(hp + 1) * 128]
                src = src.rearrange("(k p) m -> p k m", p=128)
                eng.dma_start(out=wt.rearrange("p (k m) -> p k m", k=NKT), in_=src)
                tiles[which] = wt
            wqk[hp] = tiles
            qts[hp] = sb_l.tile([128, T], bf16, tag="qt", bufs=2, name=f"qt{hp}")
            kts[hp] = sb_l.tile([128, T], bf16, tag="kt", bufs=2, name=f"kt{hp}")

        def emit_qk_group(hp, which, tck):
            wt = wqk[hp][which]
            ot = qts[hp] if which == "q" else kts[hp]
            pq = ps.tile([128, 512], f32, tag="acc", bufs=2)
            for k in range(NKT):
                nc.tensor.matmul(pq[:], wt[:, k * 128:(k + 1) * 128],
                                 xts[k][:, tck * 512:(tck + 1) * 512],
                                 start=(k == 0), stop=(k == NKT - 1))
            nc.vector.tensor_copy(ot[:, tck * 512:(tck + 1) * 512], pq[:])

        vts = [None] * NTT

        def emit_v_group(tt):
            na = _nact(tt)           # active heads at this key tile (8, 6 or 4)
            pvm = ps.tile([128, 512], f32, tag="acc", bufs=2)
            for k in range(NKT):
                nc.tensor.matmul(pvm[:, 0:64 * na],
                                 xts[k][:, tt * 128:(tt + 1) * 128],
                                 wvs[k][:, 0:64 * na],
                                 start=(k == 0), stop=(k == NKT - 1))
            vt = sb_r.tile([128, na * 65], bf16, tag=f"v{tt}", bufs=1, name=f"v{tt}")
            v3 = vt.rearrange("p (h c) -> p h c", h=na)
            for h in range(na):
                nc.vector.tensor_scalar_mul(
                    v3[:, h, 0:64], pvm[:, h * 64:(h + 1) * 64],
                    gv_sb[:, tt * HL + h:tt * HL + h + 1])
            nc.vector.tensor_copy(
                v3[:, :, 64:65],
                gv_sb[:, tt * HL:tt * HL + na].rearrange("p (h c) -> p h c", c=1))
            vts[tt] = vt

        wps = [None] * 8

        def emit_wp_dma():
            for i in range(8):  # i = hp*2 + ec
                hp, ec = i // 2, i % 2
                t = sb_l.tile([128, 512], bf16, tag="wst", bufs=8, name=f"wp{i}")
                nc.gpsimd.dma_start(
                    out=t[:],
                    in_=wpT_d[hp * 128:(hp + 1) * 128,
                              ec * 512:(ec + 1) * 512])
                wps[i] = t

        def emit_proj_group(tt, ec):
            po = ps.tile([128, 512], f32, tag="acc", bufs=2)
            for hp in range(4):
                nc.tensor.matmul(po[:], outTs[hp][:, tt * 128:(tt + 1) * 128],
                                 wps[hp * 2 + ec][:], start=(hp == 0), stop=(hp == 3))
            ob = sb_l.tile([128, 512], bf16, tag="ob", bufs=2)
            nc.vector.tensor_copy(ob[:], po[:])
            nc.sync.dma_start(out=out_d[tt * 128:(tt + 1) * 128,
                                        ec * 512:(ec + 1) * 512],
                              in_=ob[:])

        # ---------- filler singles queue ----------
        # Fill work (QKV projections, V staging, output proj) is broken into
        # SINGLE-matmul thunks and pumped between attention tiles so the PE
        # never idles while ACT (exp) runs: the PE must stay near-100% busy in
        # every 3.4us HAM window or the clock drops to 1.2 GHz.
        # Queue is FIFO in deadline order; need_by = (slot_pos, c) in
        # processing order.  Group state (psum acc tile) lives in a closure:
        # acc tag bufs=2 and FIFO consumption mean at most 2 open groups.
        SLOT_ORDER = [3, 2, 0, 1]
        lateq = []   # items: (cost_ns, need_by, thunk, gid, is_last)
        _gid = [0]
        open_gid = [None]

        def q_push(cost, need_by, fn, gid=None, last=True):
            lateq.append((cost, need_by, fn, gid, last))

        def _pop_run():
            cost, _, fn, gid, last = lateq.pop(0)
            fn()
            open_gid[0] = None if (last or gid is None) else gid
            return cost

        def close_group():
            # finish the currently open psum-acc accumulation group so a
            # non-queue acc-tag alloc (normalize's bcast) can't deadlock
            while lateq and open_gid[0] is not None:
                _pop_run()

        def qk_singles(hp, which, tck, need_by):
            st = {}
            _gid[0] += 1
            g = _gid[0]

            def mk(k):
                def f():
                    if k == 0:
                        st["pq"] = ps.tile([128, 512], f32, tag="acc", bufs=2, name=f"pq{g}")
                    nc.tensor.matmul(st["pq"][:],
                                     wqk[hp][which][:, k * 128:(k + 1) * 128],
                                     xts[k][:, tck * 512:(tck + 1) * 512],
                                     start=(k == 0), stop=(k == NKT - 1))
                return f
            for k in range(NKT):
                q_push(213, need_by, mk(k), gid=g, last=False)

            def cast():
                ot = qts[hp] if which == "q" else kts[hp]
                nc.vector.tensor_copy(ot[:, tck * 512:(tck + 1) * 512],
                                      st["pq"][:])
            q_push(0, need_by, cast, gid=g, last=True)

        def qk_push(hp, tck, need_by):
            qk_singles(hp, "q", tck, need_by)
            if tck < KCH[hp]:
                qk_singles(hp, "k", tck, need_by)

        def v_singles(tt, need_by):
            na = _nact(tt)
            st = {}
            _gid[0] += 1
            g = _gid[0]

            def mk(k):
                def f():
                    if k == 0:
                        st["pvm"] = ps.tile([128, 512], f32, tag="acc", bufs=2, name=f"pvm{g}")
                    nc.tensor.matmul(st["pvm"][:, 0:64 * na],
                                     xts[k][:, tt * 128:(tt + 1) * 128],
                                     wvs[k][:, 0:64 * na],
                                     start=(k == 0), stop=(k == NKT - 1))
                return f
            for k in range(NKT):
                q_push(27 * na, need_by, mk(k), gid=g, last=False)

            def tailf():
                vt = sb_r.tile([128, na * 65], bf16, tag=f"v{tt}", bufs=1,
                               name=f"v{tt}")
                v3 = vt.rearrange("p (h c) -> p h c", h=na)
                for h in range(na):
                    nc.vector.tensor_scalar_mul(
                        v3[:, h, 0:64], st["pvm"][:, h * 64:(h + 1) * 64],
                        gv_sb[:, tt * HL + h:tt * HL + h + 1])
                nc.vector.tensor_copy(
                    v3[:, :, 64:65],
                    gv_sb[:, tt * HL:tt * HL + na].rearrange(
                        "p (h c) -> p h c", c=1))
                vts[tt] = vt
            q_push(0, need_by, tailf, gid=g, last=True)

        def proj_singles(tt, ec, need_by):
            st = {}
            _gid[0] += 1
            g = _gid[0]

            def mk(hp):
                def f():
                    if hp == 0:
                        st["po"] = ps.tile([128, 512], f32, tag="acc", bufs=2, name=f"po{g}")
                    nc.tensor.matmul(st["po"][:],
                                     outTs[hp][:, tt * 128:(tt + 1) * 128],
                                     wps[hp * 2 + ec][:],
                                     start=(hp == 0), stop=(hp == 3))
                return f
            for hp in range(4):
                q_push(213, need_by, mk(hp), gid=g, last=False)

            def tailf():
                ob = sb_l.tile([128, 512], bf16, tag="ob", bufs=2)
                nc.vector.tensor_copy(ob[:], st["po"][:])
                nc.sync.dma_start(out=out_d[tt * 128:(tt + 1) * 128,
                                            ec * 512:(ec + 1) * 512],
                                  in_=ob[:])
            q_push(0, need_by, tailf, gid=g, last=True)

        # enqueue everything in deadline order (constraints: wqk DMA only
        # after the previous slot's qk groups; wp DMA after the last V group)
        q_push(0, (0, 1), lambda: dma_xt_chunk(2))
        qk_push(3, 1, (0, 1))
        qk_push(3, 2, (0, 2))
        q_push(0, (0, 3), lambda: dma_xt_chunk(3))
        qk_push(3, 3, (0, 3))
        v_singles(2, (1, 0))
        v_singles(3, (1, 0))
        q_push(0, (1, 0), lambda: emit_wqk_dma(2))
        qk_push(2, 0, (1, 0))
        v_singles(4, (1, 1))
        qk_push(2, 1, (1, 1))
        qk_push(2, 2, (1, 2))
        qk_push(2, 3, (1, 3))
        q_push(0, (2, 0), lambda: emit_wqk_dma(0))
        qk_push(0, 0, (2, 0))
        for tt in (5, 6, 7):
            v_singles(tt, (2, 1))
        qk_push(0, 1, (2, 1))
        for tt in (8, 9, 10, 11):
            v_singles(tt, (2, 2))
        qk_push(0, 2, (2, 2))
        qk_push(0, 3, (2, 3))
        for tt in (12, 13, 14, 15):
            v_singles(tt, (2, 3))
        q_push(0, (3, 0), emit_wp_dma)
        q_push(0, (3, 0), lambda: emit_wqk_dma(1))
        qk_push(1, 0, (3, 0))
        qk_push(1, 1, (3, 1))
        qk_push(1, 2, (3, 2))
        qk_push(1, 3, (3, 3))

        debt = [0.0]

        def pump(ns):
            debt[0] = min(debt[0] + ns, 3000.0)
            while lateq and debt[0] >= lateq[0][0]:
                debt[0] -= _pop_run()

        def drain(upto):
            while lateq and lateq[0][1] <= upto:
                _pop_run()

        # ---------- preloop ----------
        nc.gpsimd.dma_start(out=gv_sb[:], in_=eb_d[:])
        nc.gpsimd.dma_start(out=ones_t[:], in_=ones_d[:, 0:64].bitcast(f32r))
        emit_wqk_dma(3)
        dma_xt_half(0, 1)
        dma_xt_chunk(1)
        for tt in range(2):
            emit_v_group(tt)
        emit_qk_group(3, "q", 0)
        emit_qk_group(3, "k", 0)

        # ---------- attention ----------
        # normalize: outT[d, i] = pv[d, i] * (1 / pv[64, i]).
        # approx recip needs a base-partition-0 AP (the custom DVE op
        # misreads offset APs); rows 0:64 are don't-care.
        def norm_dve_part(pv0, pv1):
            # reciprocal chain only (DVE): runs right at the pv stop so the
            # result is ready when the deferred PE part fires next chunk
            rrrs = []
            for pvx in (pv0, pv1):
                rr = sb_l.tile([65, 512], f32, tag="rr", bufs=1)
                nc.vector.reciprocal_approx_fast(rr[0:65, :], pvx[0:65, :])
                rrr = sb_l.tile([65, 512], f32r, tag="rrr", bufs=2)
                nc.vector.tensor_copy(rrr[64:65, :], rr[64:65, :])
                rrrs.append(rrr)
            return rrrs

        def norm_pe_part(oT, c, pv0, pv1, rrrs):
            for par, pvx in ((0, pv0), (1, pv1)):
                bp = ps.tile([64, 512], f32, tag="acc", bufs=2)
                nc.tensor.matmul(bp[0:64, :], ones_t[64:65, 0:64],
                                 rrrs[par][64:65, :], start=True, stop=True)
                bb = sb_l.tile([64, 512], bf16, tag="bb", bufs=2)
                nc.scalar.copy(bb[:], bp[0:64, :])
                if par == 0:
                    nc.vector.tensor_mul(oT[0:64, c * 512:(c + 1) * 512],
                                         pvx[0:64, :], bb[:])
                else:
                    tm = sb_l.tile([64, 512], bf16, tag="tm", bufs=1)
                    nc.vector.tensor_mul(tm[:], pvx[0:64, :], bb[:])
                    nc.sync.dma_start(out=oT[64:128, c * 512:(c + 1) * 512],
                                      in_=tm[:])

        pend = [None]

        for spos, hp in enumerate(SLOT_ORDER):
            drain((spos, 0))   # ensures this slot's wqk DMA (and tiles) exist
            qt, kt = qts[hp], kts[hp]
            oT = sb_r.tile([128, T], bf16, tag=f"ot{hp}", bufs=1, name=f"ot{hp}")
            outTs[hp] = oT
            for c in range(4):
                drain((spos, c))
                pv0 = ps.tile([128, 512], f32, tag="pv", bufs=2)
                pv1 = ps.tile([128, 512], f32, tag="pv", bufs=2)
                njt = min(4 * c + 4, SLOT_M[hp])
                h0off = 2 * hp * 65

                def emit_pv(jt, cw):
                    p01 = p01s[jt % 2]
                    nc.tensor.matmul(pv0[0:65, 512 - cw:512],
                                     vts[jt][:, h0off:h0off + 65],
                                     p01[:, 0:cw],
                                     start=(jt == 0), stop=(jt == njt - 1))
                    nc.tensor.matmul(pv1[0:65, 512 - cw:512],
                                     vts[jt][:, h0off + 65:h0off + 130],
                                     p01[:, 512:512 + cw],
                                     start=(jt == 0), stop=(jt == njt - 1))

                p01s = [None, None]
                cws = [0] * njt
                for jt in range(njt):
                    r = jt - 4 * c
                    # bf16 operands have no min-free-dim penalty: use the
                    # exact unmasked width per diagonal tile
                    cw = 512 - 128 * r if r > 0 else 512
                    mw = 128
                    ioff = c * 512 + (512 - cw)
                    cws[jt] = cw
                    s01 = ps.tile([128, 1024], f32, tag="s", bufs=2)
                    nc.tensor.matmul(s01[:, 0:cw], kt[0:64, jt * 128:(jt + 1) * 128],
                                     qt[0:64, ioff:ioff + cw], start=True, stop=True)
                    nc.tensor.matmul(s01[:, 512:512 + cw],
                                     kt[64:128, jt * 128:(jt + 1) * 128],
                                     qt[64:128, ioff:ioff + cw], start=True, stop=True)
                    p01 = sb_l.tile([128, 1024], bf16, tag="pt", bufs=2)
                    p01s[jt % 2] = p01
                    s3 = s01.rearrange("p (h m) -> p h m", h=2)
                    p3 = p01.rearrange("p (h m) -> p h m", h=2)
                    nc.scalar.activation(p3[:, :, 0:cw], s3[:, :, 0:cw], Exp)
                    if r >= 0:
                        # zero the j > i region at the head of the window:
                        # keep where (i - j) = (m - (mw - 128)) - pj >= 0
                        for off in (0, 512):
                            nc.gpsimd.affine_select(
                                out=p01[:, off:off + mw], in_=p01[:, off:off + mw],
                                compare_op=mybir.AluOpType.is_ge, fill=0.0,
                                base=-(mw - 128), pattern=[[1, mw]],
                                channel_multiplier=-1)
                    # software pipeline: PV runs one tile behind S, with
                    # pumped filler singles covering the exp latency.
                    # deficit per tile ~= ACT time - attention PE time (warm)
                    if jt > 0:
                        if (jt == min(2, njt - 1) and pend[0] is not None):
                            close_group()
                            p_hp, p_args = pend[0]
                            norm_pe_part(*p_args)
                            pend[0] = None
                            if p_hp == 1:   # unlocks proj for that chunk
                                for tt in range(4 * p_args[1],
                                                4 * p_args[1] + 4):
                                    for ec in range(2):
                                        proj_singles(tt, ec, (3, 9))
                        pump((172 + 2 * cw) / 1.2 - 1.25 * cw
                             + (250 if r >= 0 else 0))
                        emit_pv(jt - 1, cws[jt - 1])
                pump(600)
                emit_pv(njt - 1, cws[njt - 1])
                rrrs = norm_dve_part(pv0, pv1)
                pend[0] = (hp, (oT, c, pv0, pv1, rrrs))
        close_group()
        norm_pe_part(*pend[0][1])
        pend[0] = None
        for tt in range(12, 16):
            for ec in range(2):
                proj_singles(tt, ec, (3, 9))
        while lateq:
            _pop_run()

    nc.finalize()
    return nc


def _get_nc():
    global _NC_CACHE
    if _NC_CACHE is None:
        _NC_CACHE = _build_nc()
    return _NC_CACHE


def _slopes():
    start = 2.0 ** (-(2.0 ** (-(math.log2(H) - 3.0))))
    return np.array([start * start ** i for i in range(H)], dtype=np.float64)


def _host_prep(x, Wq, Aq, Bq, Wk, Ak, Bk, Wv, Av, Bv, Wp):
    f8 = np.float64
    weff = {}
    for nm, W, A, B in (("q", Wq, Aq, Bq), ("k", Wk, Ak, Bk), ("v", Wv, Av, Bv)):
        weff[nm] = (W.astype(f8) + 2.0 * (A.astype(f8) @ B.astype(f8)))
    weff["q"] = weff["q"] / math.sqrt(DH)          # fold 1/sqrt(dh) into q weights
    slopes = _slopes()

    jj = np.arange(T, dtype=np.float64).reshape(16, 128).T   # [pj, tt] -> j

    import ml_dtypes
    bf = ml_dtypes.bfloat16

    in_maps = []
    for b in range(4):
        xT = np.ascontiguousarray(x[b].T).astype(bf)
        for half in range(2):
            heads = HEADS_HALF[half]
            rows = np.concatenate([np.arange(h * 64, (h + 1) * 64) for h in heads])
            # gv[pj, tt*8 + hl] = exp(-(slope_h * j + C)), j = tt*128 + pj
            gv = np.stack([np.exp(-(slopes[heads[hl]] * jj + CB))
                           for hl in range(HL)], axis=2)   # [128, 16, 8]
            gv = gv.reshape(128, 16 * HL).astype(np.float32)
            in_maps.append({
                "xT": xT,
                "wqT": np.ascontiguousarray(weff["q"][rows].T).astype(bf),
                "wkT": np.ascontiguousarray(weff["k"][rows].T).astype(bf),
                "wvT": np.ascontiguousarray(weff["v"][rows].T).astype(bf),
                "wpT": np.ascontiguousarray(Wp[:, rows].T).astype(bf),
                "ebias": gv,
                "onesd": np.ones((128, 128), dtype=np.float32),
            })
    return in_maps


def run(inputs, trace=False):
    nc = _get_nc()
    inputs = {k: np.asarray(v, dtype=np.float32) for k, v in inputs.items()}
    in_maps = _host_prep(**inputs)
    res = run_bass_kernel_spmd(nc, in_maps, list(range(8)), trace=trace)
    outs = [np.asarray(res.results[i]["out"]).astype(np.float32)
            for i in range(8)]
    full = np.stack([outs[2 * b] + outs[2 * b + 1] for b in range(4)])
    return full.astype(np.float32), res


def kernel(**inputs):
    full, _ = run(inputs, trace=False)
    return full
